# revision 30
# baseline (speedup 1.0000x reference)
"""Multi-head attention (B=4, S=2048, D=256, H=4) on 8 trn2 NeuronCores.

Sharding: core c handles batch b = c//2 and query half qh = c%2 (1024
queries), all 4 heads, full 2048 keys.  The host rolls x[b].T by
-qh*1024 columns so every core's queries sit at columns 0-1023 of its
xT input (key order is free: softmax+sum over keys is permutation
invariant as long as the mask bias is rolled identically).  This keeps
the SPMD program core-agnostic and avoids a separate xq input.

Per-core dataflow (scores kept transposed: [keys, queries]):
  QT = WQT.T-chunks @ xT[:, :1024]  -> Q.T [256(feat), 1024(q)]
  KT = WKT.T-chunks @ xT            -> K.T [256(feat), 2048(k)]
  V  = xT-chunks.T  @ WVT           -> V_aug [2048(k), 4, 65] (ones col)
  per head pair p, query half f, key tile kt (128 keys):
     S.T[kt, q] = KT_h-slices.T @ QT-slices  (2 heads row-packed in PE)
     E.T        = exp(S.T * scale + mask_bias[key])      (ScalarE)
     cd_h      += V_aug_h.T @ E.T   (rows 0-63 = ctx.T, row 64 = den;
                                     one PSUM bank per head, accumulated
                                     over the 16 key tiles)
  after each section: cd evicted to SBUF; den row transposed to
  partitions via 8 single-row matmuls -> [128,8] -> DVE reciprocal.
  out tile m: per-head matmuls ps4[:,h,:] = ctx_h.T-chunk @ WOT_h, then
  out = sum_h ps4[:,h,:] * (1/den_h) via DVE per-partition-scalar
  multiply-adds (normalization folded into the output combine, which is
  valid per head since each head's 1/den[h,q] scales output rows).

The single-partition [1,1024] DVE reciprocal of the previous design
(6.5us, head-of-line blocking the in-order DVE FIFO and stalling the PE
long enough to re-trigger the HAM clock throttle) is gone; all
reciprocals run on [128,8] tiles.

Matmul operands are float32r (TF32-like, 1 PE cycle/col for N>=256).
fp32r matmuls must write PSUM at partition offset 0, which all dsts
here do.  Input DMAs are split into 512-col pieces issued on the sync
and gpsimd queues in consumption order so projections and the first
attention section start while the bulk of xT is still in flight.
"""

import sys

for _p in ("/opt/trn_rl_repo",):
    if _p not in sys.path:
        sys.path.insert(0, _p)

import numpy as np

B, S, D, H, HD = 4, 2048, 256, 4, 64
SCALE = HD**-0.5
NCORES = 8
QS = S // 2  # queries per core
QH = QS // 2  # query half (one psum bank wide per head)
P = 128
NKT = S // P  # 16 key tiles

_cache = {}


def _build_nc():
    import concourse.mybir as mybir
    from concourse import bacc
    from concourse.tile import TileContext

    f32 = mybir.dt.float32
    f32r = mybir.dt.float32r
    Exp = mybir.ActivationFunctionType.Exp
    Copy = mybir.ActivationFunctionType.Copy
    Alu = mybir.AluOpType

    nc = bacc.Bacc("TRN2", target_bir_lowering=False, debug=False)

    xT_d = nc.dram_tensor("xT", [D, S], f32, kind="ExternalInput")
    wqt_d = nc.dram_tensor("wqt", [D, D], f32, kind="ExternalInput")
    wkt_d = nc.dram_tensor("wkt", [D, D], f32, kind="ExternalInput")
    wvt_d = nc.dram_tensor("wvt", [D, D], f32, kind="ExternalInput")
    wot_d = nc.dram_tensor("wot", [D, D], f32, kind="ExternalInput")
    bias_d = nc.dram_tensor("bias", [P, NKT], f32, kind="ExternalInput")
    out_d = nc.dram_tensor("out", [QS, D], f32, kind="ExternalOutput")

    with TileContext(nc) as tc:
        with (
            tc.tile_pool(name="const", bufs=1) as const,
            tc.tile_pool(name="big", bufs=1) as big,
            tc.tile_pool(name="et", bufs=6) as etp,
            tc.tile_pool(name="small", bufs=2) as small,
            tc.tile_pool(name="psA", bufs=3, space="PSUM") as psA,
            tc.tile_pool(name="psCD", bufs=1, space="PSUM") as psCD,
        ):
            # ---- input DMAs, split by first consumption and ordered by
            # priority on two hardware queues (sync: Q/K path, gpsimd: V/
            # later xT pieces/O path).  The scalar engine issues nothing so
            # the exp ACTIVATEs never queue behind a DMA descriptor gen. ----
            w_sb = {}
            for nm, dram, eng in (
                ("wqt", wqt_d, nc.sync),
                ("wkt", wkt_d, nc.sync),
                ("wvt", wvt_d, nc.gpsimd),
            ):
                wt = const.tile([P, 2, D], f32r, name=f"w_{nm}", tag=f"w_{nm}")
                eng.dma_start(
                    out=wt, in_=dram.rearrange("(c p) e -> p c e", p=P).bitcast(f32r)
                )
                w_sb[nm] = wt
            bias_sb = const.tile([P, NKT], f32)
            nc.gpsimd.dma_start(out=bias_sb, in_=bias_d[:, :])

            xT_sb = []
            for c in range(2):
                xt = big.tile([P, S], f32r, name=f"xT{c}", tag=f"xT{c}")
                xT_sb.append(xt)
            # pieces in consumption order: q0 feeds Q-proj(f=0) + K tiles
            # 0-3 + V tiles 0-3; q1 feeds Q-proj(f=1) + K/V tiles 4-7; ...
            for q, eng in ((0, nc.sync), (1, nc.sync), (2, nc.gpsimd), (3, nc.gpsimd)):
                for c in range(2):
                    eng.dma_start(
                        out=xT_sb[c][:, q * 512 : (q + 1) * 512],
                        in_=xT_d[c * P : (c + 1) * P, q * 512 : (q + 1) * 512].bitcast(
                            f32r
                        ),
                    )
            # W_O.T grouped per head: [64, 4, 256] so each head's contraction
            # chunk starts at partition 0.
            wot_sb = const.tile([64, 4, D], f32r, name="w_wot", tag="w_wot")
            nc.gpsimd.dma_start(
                out=wot_sb, in_=wot_d.rearrange("(h p) e -> p h e", p=64).bitcast(f32r)
            )

            # ---- constants ----
            ones4 = const.tile([P, 4], f32)
            nc.vector.memset(ones4, 1.0)
            one1 = const.tile([P, 2], f32r)
            nc.vector.tensor_copy(one1, ones4[:, 0:2])

            # PE warm-up: the HAM clock gate releases (K=4/8 -> 8/8) only
            # after ~10us of sustained PE activity.  Burn tiny matmuls on
            # const data while the input DMAs stream so the gate is open
            # when the real work starts.  They write the psCD slot, which
            # is unused until cd00; the WAR dep serializes warm-up before
            # the first cd matmul (all long done by then).
            ps_warm = psCD.tile([2, 2], f32, name="ps_warm", tag="psCD")
            for _ in range(40):
                nc.tensor.matmul(
                    ps_warm[0:2, 0:2], one1, one1, start=True, stop=True
                )

            QT_sb = [None, None]
            KT_sb = [None, None]
            V_sb = [None] * NKT
            cd_sb = {}
            r_sb = {}
            for p in range(2):
                for f in range(2):
                    r_sb[(p, f)] = big.tile(
                        [P, 16], f32, name=f"r{p}{f}", tag=f"r{p}{f}"
                    )

            def qt_proj(m, n):
                # QT_sb[m][:, n*512:(n+1)*512] (feature rows m*128..)
                if QT_sb[m] is None:
                    QT_sb[m] = big.tile([P, QS], f32r, name=f"QT{m}", tag=f"QT{m}")
                ps = psA.tile([P, 512], f32, name="ps_q", tag="psA")
                for c in range(2):
                    nc.tensor.matmul(
                        ps[:, :],
                        w_sb["wqt"][:, c, m * P : (m + 1) * P],
                        xT_sb[c][:, n * 512 : (n + 1) * 512],
                        start=(c == 0),
                        stop=(c == 1),
                    )
                nc.vector.tensor_copy(QT_sb[m][:, n * 512 : (n + 1) * 512], ps)

            def kt_proj(m, q):
                # KT_sb[m][:, q*512:(q+1)*512]
                if KT_sb[m] is None:
                    KT_sb[m] = big.tile([P, S], f32r, name=f"KT{m}", tag=f"KT{m}")
                ps = psA.tile([P, 512], f32, name="ps_k", tag="psA")
                for c in range(2):
                    nc.tensor.matmul(
                        ps[:, :],
                        w_sb["wkt"][:, c, m * P : (m + 1) * P],
                        xT_sb[c][:, q * 512 : (q + 1) * 512],
                        start=(c == 0),
                        stop=(c == 1),
                    )
                nc.vector.tensor_copy(KT_sb[m][:, q * 512 : (q + 1) * 512], ps)

            def v_proj(mt):
                # V_aug in fp8, kt-PAIR layout for DoubleRow cd matmuls:
                # V8[j][:, i, h, 0:64] = V of key tile 2j+i, head h; col 64 is
                # a ones col (whose cd-matmul row is the softmax denominator).
                vt = big.tile([P, 4, 65], f32r, name=f"V{mt}", tag=f"V{mt}")
                ps = psA.tile([P, 512], f32, name="ps_v", tag="psA")
                for c in range(2):
                    nc.tensor.matmul(
                        ps[:, :D],
                        xT_sb[c][:, mt * P : (mt + 1) * P],
                        w_sb["wvt"][:, c, :],
                        start=(c == 0),
                        stop=(c == 1),
                    )
                nc.vector.tensor_copy(
                    vt[:, :, 0:64], ps[:, :D].rearrange("p (h e) -> p h e", h=4)
                )
                nc.vector.tensor_copy(vt[:, :, 64], ones4)
                V_sb[mt] = vt

            def kt_loop(p, f, inject=None):
                # rows 0-63: ctx.T for head 2p+h2; row 64: denominator.
                # One bank per head (h2 chooses the 512-col half).  The exp
                # tiles are written in fp8 kt-pair layout so each cd matmul
                # contracts TWO key tiles per pass (fp8 DoubleRow, 0.5
                # cycles/col): rhs free dims (2, 512) pick the kt-parity
                # halves, lhsT free dims (2, 65) the matching V slices.
                # Software-pipelined emission: the PE engine is in-order, so
                # cd(kt) is emitted only after scores(kt+2).  A cd matmul
                # blocked on its exp ACT (or, at kt=0, on the previous
                # section's cdsb eviction COPY) then always has two kt-steps
                # of independent scores work queued ahead of it, keeping the
                # PE busy (HAM clock gate stays released).
                ps_cd = psCD.tile([65, 1024], f32, name="ps_cd", tag="psCD")
                ets = {}

                def scores_act(kt):
                    ps_s = psA.tile([P, 1024], f32, name="ps_s", tag="psA")
                    for h2 in range(2):
                        nc.tensor.matmul(
                            ps_s[:, h2 * 512 : h2 * 512 + QH],
                            KT_sb[p][64 * h2 : 64 * h2 + 64, kt * P : (kt + 1) * P],
                            QT_sb[p][64 * h2 : 64 * h2 + 64, f * QH : (f + 1) * QH],
                            start=True,
                            stop=True,
                            tile_position=(64 * h2, 0),
                        )
                    et = etp.tile([P, 1024], f32r, name="et", tag="et")
                    nc.scalar.activation(
                        et, ps_s, Exp, bias=bias_sb[:, kt : kt + 1], scale=SCALE
                    )
                    ets[kt] = et

                scores_act(0)
                scores_act(1)
                for kt in range(NKT):
                    if kt + 2 < NKT:
                        scores_act(kt + 2)
                    et = ets.pop(kt)
                    for h2 in range(2):
                        h = 2 * p + h2
                        nc.tensor.matmul(
                            ps_cd[0:65, h2 * 512 : h2 * 512 + QH],
                            V_sb[kt][:, h, :],
                            et[:, h2 * 512 : h2 * 512 + QH],
                            start=(kt == 0),
                            stop=(kt == NKT - 1),
                        )
                    if inject and kt in inject:
                        inject[kt]()
                return ps_cd

            def finish_cd(p, f, ps_cd):
                # Evict ctx+den to SBUF right at section end, freeing the
                # PSUM slot for the next-but-one section.
                cdsb = big.tile([65, 1024], f32r, name=f"cd{p}{f}", tag=f"cd{p}{f}")
                nc.vector.tensor_copy(cdsb, ps_cd)
                cd_sb[(p, f)] = cdsb

            def den_recip(p, f):
                # Transpose the [1,1024] den row into partitions via 8
                # single-row matmuls, then one cheap [128,8] reciprocal.
                # col layout: h2*4 + q128 (q128 = 128-query block in half f).
                cdsb = cd_sb[(p, f)]
                ps_den = psA.tile([P, 16], f32, name="ps_den", tag="psA")
                for t in range(8):
                    h2, qq = t // 4, t % 4
                    # 2 duplicate output cols: fp32r ISA needs even free counts
                    nc.tensor.matmul(
                        ps_den[:, 2 * t : 2 * t + 2],
                        cdsb[64:65, h2 * 512 + qq * P : h2 * 512 + (qq + 1) * P],
                        one1[64:65, 0:2],
                        start=True,
                        stop=True,
                    )
                nc.vector.reciprocal(r_sb[(p, f)][:, 0:16], ps_den[:, 0:16])

            def r_ap(m, h):
                f, qq = m // 4, m % 4
                c = 2 * ((h % 2) * 4 + qq)
                return r_sb[(h // 2, f)][:, c : c + 1]

            def oproj_mm2(m, hpair):
                # per-head matmuls for heads (2*hpair, 2*hpair+1) of out tile m
                f, qq = m // 4, m % 4
                ps2 = psA.tile([P, 2, D], f32, name="ps2", tag="psA")
                for h2 in range(2):
                    h = 2 * hpair + h2
                    nc.tensor.matmul(
                        ps2[:, h2, :],
                        cd_sb[(hpair, f)][
                            0:64, h2 * 512 + qq * P : h2 * 512 + (qq + 1) * P
                        ],
                        wot_sb[:, h, :],
                        start=True,
                        stop=True,
                    )
                return ps2

            accA = {}

            def oproj_a(m):
                # tail tiles, heads 0-1: runs while the last section's cd
                # PSUM is still being evicted.  ScalarE (idle after the last
                # exp) does the 1/den scaling, GpSimd the add — DVE stays
                # free for the eviction COPY.
                ps2 = oproj_mm2(m, 0)
                t0 = small.tile([P, D], f32, name="tA", tag="tA")
                t1 = small.tile([P, D], f32, name="tA", tag="tA")
                nc.scalar.activation(t0, ps2[:, 0, :], Copy, scale=r_ap(m, 0))
                nc.scalar.activation(t1, ps2[:, 1, :], Copy, scale=r_ap(m, 1))
                acc = small.tile([P, D], f32, name=f"accA{m}", tag=f"accA{m}", bufs=1)
                nc.gpsimd.tensor_add(acc, t0, t1)
                accA[m] = acc

            def oproj_b(m):
                # tail tiles, heads 2-3 + final combine + output DMA
                ps2 = oproj_mm2(m, 1)
                t = small.tile([P, D], f32, name="acc", tag="acc")
                nc.vector.scalar_tensor_tensor(
                    t, ps2[:, 0, :], r_ap(m, 2), accA[m], Alu.mult, Alu.add
                )
                ot = small.tile([P, D], f32, name="ot", tag="ot", bufs=3)
                nc.vector.scalar_tensor_tensor(
                    ot, ps2[:, 1, :], r_ap(m, 3), t, Alu.mult, Alu.add
                )
                # alternate output queues so the tail DMA issues overlap
                eng = nc.sync if m % 2 == 0 else nc.gpsimd
                eng.dma_start(out=out_d[m * P : (m + 1) * P, :], in_=ot)

            def oproj(m):
                # out tile m (queries m*128..): per-head matmul (no accum
                # across heads), then normalization folded into the combine:
                # out = sum_h ps4[:,h,:] * (1/den_h) with per-partition
                # scalars from r_sb.
                f, qq = m // 4, m % 4
                ps4 = psA.tile([P, 4, D], f32, name="ps4", tag="psA")
                for h in range(H):
                    p, h2 = h // 2, h % 2
                    nc.tensor.matmul(
                        ps4[:, h, :],
                        cd_sb[(p, f)][0:64, h2 * 512 + qq * P : h2 * 512 + (qq + 1) * P],
                        wot_sb[:, h, :],
                        start=True,
                        stop=True,
                    )

                acc = small.tile([P, D], f32, name="acc", tag="acc")
                nc.vector.tensor_scalar_mul(acc, ps4[:, 0, :], r_ap(m, 0))
                for h in range(1, H):
                    dst = (
                        small.tile([P, D], f32, name="acc", tag="acc")
                        if h < H - 1
                        else small.tile([P, D], f32, name="ot", tag="ot", bufs=3)
                    )
                    nc.vector.scalar_tensor_tensor(
                        dst, ps4[:, h, :], r_ap(m, h), acc, Alu.mult, Alu.add
                    )
                    acc = dst
                nc.sync.dma_start(out=out_d[m * P : (m + 1) * P, :], in_=acc)

            # ---- prologue: only what section (0,0) needs immediately ----
            qt_proj(0, 0)
            kt_proj(0, 0)
            for mt in range(3):
                v_proj(mt)

            # section (0,0): stream V tiles 2 steps ahead of their cd-use;
            # remaining K columns and pair-1 projections fill later steps.
            # NOTE: with the software-pipelined kt_loop, scores_act(j) is
            # EMITTED at loop iteration j-2; any injection producing KT
            # columns for score tile j must sit at kt <= j-3 (Tile deps
            # follow emission order).
            inj00 = {
                1: lambda: (v_proj(3), kt_proj(0, 1)),
                2: lambda: v_proj(4),
                3: lambda: v_proj(5),
                4: lambda: v_proj(6),
                5: lambda: (v_proj(7), kt_proj(0, 2)),
                6: lambda: v_proj(8),
                7: lambda: v_proj(9),
                8: lambda: (v_proj(10), kt_proj(0, 3)),
                9: lambda: v_proj(11),
                10: lambda: v_proj(12),
                11: lambda: (v_proj(13), qt_proj(1, 0)),
                12: lambda: v_proj(14),
                13: lambda: v_proj(15),
                14: lambda: kt_proj(1, 0),
            }
            cd00 = kt_loop(0, 0, inj00)
            finish_cd(0, 0, cd00)
            cd10 = kt_loop(
                1,
                0,
                {
                    1: lambda: kt_proj(1, 1),
                    3: lambda: qt_proj(0, 1),
                    5: lambda: kt_proj(1, 2),
                    7: lambda: den_recip(0, 0),
                    9: lambda: kt_proj(1, 3),
                },
            )
            finish_cd(1, 0, cd10)
            cd01 = kt_loop(
                0, 1, {3: lambda: qt_proj(1, 1), 6: lambda: den_recip(1, 0)}
            )
            finish_cd(0, 1, cd01)
            cd11 = kt_loop(
                1,
                1,
                {
                    3: lambda: oproj(0),
                    5: lambda: oproj(1),
                    7: lambda: den_recip(0, 1),
                    9: lambda: oproj(2),
                    11: lambda: oproj(3),
                },
            )
            finish_cd(1, 1, cd11)
            for m in range(4, 8):
                oproj_a(m)
            den_recip(1, 1)
            for m in range(4, 8):
                oproj_b(m)

    nc.compile()
    return nc


def _get_nc():
    if "nc" not in _cache:
        _cache["nc"] = _build_nc()
    return _cache["nc"]


def make_in_maps(x, W_Q, W_K, W_V, W_O, mask):
    wqt = np.ascontiguousarray(W_Q.T).astype(np.float32)
    wkt = np.ascontiguousarray(W_K.T).astype(np.float32)
    wvt = np.ascontiguousarray(W_V.T).astype(np.float32)
    wot = np.ascontiguousarray(W_O.T).astype(np.float32)
    in_maps = []
    for c in range(NCORES):
        b, qh = c // 2, c % 2
        xT_b = np.asarray(x[b]).T.astype(np.float32)
        xT_roll = np.ascontiguousarray(np.roll(xT_b, -qh * QS, axis=1))
        bias = np.where(np.asarray(mask[b]) == 0, -1e30, 0.0).astype(np.float32)
        bias = np.roll(bias, -qh * QS)
        bias = np.ascontiguousarray(bias.reshape(NKT, P).T)
        in_maps.append(
            {
                "xT": xT_roll,
                "wqt": wqt,
                "wkt": wkt,
                "wvt": wvt,
                "wot": wot,
                "bias": bias,
            }
        )
    return in_maps


def gather(results):
    out = np.empty((B, S, D), np.float32)
    for c in range(NCORES):
        b, qh = c // 2, c % 2
        out[b, qh * QS : (qh + 1) * QS, :] = results[c]["out"]
    return out


def kernel(x, W_Q, W_K, W_V, W_O, mask):
    from concourse.bass_utils import run_bass_kernel_spmd

    nc = _get_nc()
    in_maps = make_in_maps(x, W_Q, W_K, W_V, W_O, mask)
    res = run_bass_kernel_spmd(nc, in_maps, core_ids=list(range(NCORES)))
    return gather(res.results)


# revision 32
# speedup vs baseline: 1.0493x; 1.0493x over previous
"""Multi-head attention (B=4, S=2048, D=256, H=4) on 8 trn2 NeuronCores.

Sharding: core c handles batch b = c//2 and query half qh = c%2 (1024
queries), all 4 heads, full 2048 keys.  The host rolls x[b].T by
-qh*1024 columns so every core's queries sit at columns 0-1023 of its
xT input (key order is free: softmax+sum over keys is permutation
invariant as long as the mask bias is rolled identically).  This keeps
the SPMD program core-agnostic and avoids a separate xq input.

Per-core dataflow (scores kept transposed: [keys, queries]):
  QT = WQT.T-chunks @ xT[:, :1024]  -> Q.T [256(feat), 1024(q)]
  KT = WKT.T-chunks @ xT            -> K.T [256(feat), 2048(k)]
  V  = xT-chunks.T  @ WVT           -> V_aug [2048(k), 4, 65] (ones col)
  per head pair p, query half f, key tile kt (128 keys):
     S.T[kt, q] = KT_h-slices.T @ QT-slices  (2 heads row-packed in PE)
     E.T        = exp(S.T * scale + mask_bias[key])      (ScalarE)
     cd_h      += V_aug_h.T @ E.T   (rows 0-63 = ctx.T, row 64 = den;
                                     one PSUM bank per head, accumulated
                                     over the 16 key tiles)
  after each section: cd evicted to SBUF; den row transposed to
  partitions via 8 single-row matmuls -> [128,8] -> DVE reciprocal.
  out tile m: per-head matmuls ps4[:,h,:] = ctx_h.T-chunk @ WOT_h, then
  out = sum_h ps4[:,h,:] * (1/den_h) via DVE per-partition-scalar
  multiply-adds (normalization folded into the output combine, which is
  valid per head since each head's 1/den[h,q] scales output rows).

The single-partition [1,1024] DVE reciprocal of the previous design
(6.5us, head-of-line blocking the in-order DVE FIFO and stalling the PE
long enough to re-trigger the HAM clock throttle) is gone; all
reciprocals run on [128,8] tiles.

Matmul operands are float32r (TF32-like, 1 PE cycle/col for N>=256).
fp32r matmuls must write PSUM at partition offset 0, which all dsts
here do.  Input DMAs are split into 512-col pieces issued on the sync
and gpsimd queues in consumption order so projections and the first
attention section start while the bulk of xT is still in flight.
"""

import sys

for _p in ("/opt/trn_rl_repo",):
    if _p not in sys.path:
        sys.path.insert(0, _p)

import numpy as np

B, S, D, H, HD = 4, 2048, 256, 4, 64
SCALE = HD**-0.5
NCORES = 8
QS = S // 2  # queries per core
QH = QS // 2  # query half (one psum bank wide per head)
P = 128
NKT = S // P  # 16 key tiles

_cache = {}


def _build_nc():
    import concourse.mybir as mybir
    from concourse import bacc
    from concourse.tile import TileContext

    f32 = mybir.dt.float32
    f32r = mybir.dt.float32r
    Exp = mybir.ActivationFunctionType.Exp
    Copy = mybir.ActivationFunctionType.Copy
    Alu = mybir.AluOpType

    nc = bacc.Bacc("TRN2", target_bir_lowering=False, debug=False)

    xT_d = nc.dram_tensor("xT", [D, S], f32, kind="ExternalInput")
    wqt_d = nc.dram_tensor("wqt", [D, D], f32, kind="ExternalInput")
    wkt_d = nc.dram_tensor("wkt", [D, D], f32, kind="ExternalInput")
    wvt_d = nc.dram_tensor("wvt", [D, D], f32, kind="ExternalInput")
    wot_d = nc.dram_tensor("wot", [D, D], f32, kind="ExternalInput")
    bias_d = nc.dram_tensor("bias", [P, NKT], f32, kind="ExternalInput")
    out_d = nc.dram_tensor("out", [QS, D], f32, kind="ExternalOutput")

    with TileContext(nc) as tc:
        with (
            tc.tile_pool(name="const", bufs=1) as const,
            tc.tile_pool(name="big", bufs=1) as big,
            tc.tile_pool(name="et", bufs=6) as etp,
            tc.tile_pool(name="small", bufs=2) as small,
            tc.tile_pool(name="psA", bufs=3, space="PSUM") as psA,
            tc.tile_pool(name="psCD", bufs=1, space="PSUM") as psCD,
        ):
            # ---- input DMAs, split by first consumption and ordered by
            # priority on two hardware queues (sync: Q/K path, gpsimd: V/
            # later xT pieces/O path).  The scalar engine issues nothing so
            # the exp ACTIVATEs never queue behind a DMA descriptor gen. ----
            w_sb = {}
            for nm, dram, eng in (
                ("wqt", wqt_d, nc.sync),
                ("wkt", wkt_d, nc.sync),
                ("wvt", wvt_d, nc.gpsimd),
            ):
                wt = const.tile([P, 2, D], f32r, name=f"w_{nm}", tag=f"w_{nm}")
                eng.dma_start(
                    out=wt, in_=dram.rearrange("(c p) e -> p c e", p=P).bitcast(f32r)
                )
                w_sb[nm] = wt
            bias_sb = const.tile([P, NKT], f32)
            nc.gpsimd.dma_start(out=bias_sb, in_=bias_d[:, :])

            xT_sb = []
            for c in range(2):
                xt = big.tile([P, S], f32r, name=f"xT{c}", tag=f"xT{c}")
                xT_sb.append(xt)
            # pieces in consumption order: q0 feeds Q-proj(f=0) + K tiles
            # 0-3 + V tiles 0-3; q1 feeds Q-proj(f=1) + K/V tiles 4-7; ...
            for q, eng in ((0, nc.sync), (1, nc.sync), (2, nc.gpsimd), (3, nc.gpsimd)):
                for c in range(2):
                    eng.dma_start(
                        out=xT_sb[c][:, q * 512 : (q + 1) * 512],
                        in_=xT_d[c * P : (c + 1) * P, q * 512 : (q + 1) * 512].bitcast(
                            f32r
                        ),
                    )
            # W_O.T grouped per head: [64, 4, 256] so each head's contraction
            # chunk starts at partition 0.
            wot_sb = const.tile([64, 4, D], f32r, name="w_wot", tag="w_wot")
            nc.gpsimd.dma_start(
                out=wot_sb, in_=wot_d.rearrange("(h p) e -> p h e", p=64).bitcast(f32r)
            )

            # ---- constants ----
            ones4 = const.tile([P, 4], f32)
            nc.vector.memset(ones4, 1.0)
            one1 = const.tile([P, 2], f32r)
            nc.vector.tensor_copy(one1, ones4[:, 0:2])



            QT_sb = [None, None]
            KT_sb = [None, None]
            V_sb = [None] * NKT
            cd_sb = {}
            r_sb = {}
            for p in range(2):
                for f in range(2):
                    r_sb[(p, f)] = big.tile(
                        [P, 16], f32, name=f"r{p}{f}", tag=f"r{p}{f}"
                    )

            def qt_proj(m, n):
                # QT_sb[m][:, n*512:(n+1)*512] (feature rows m*128..)
                if QT_sb[m] is None:
                    QT_sb[m] = big.tile([P, QS], f32r, name=f"QT{m}", tag=f"QT{m}")
                ps = psA.tile([P, 512], f32, name="ps_q", tag="psA")
                for c in range(2):
                    nc.tensor.matmul(
                        ps[:, :],
                        w_sb["wqt"][:, c, m * P : (m + 1) * P],
                        xT_sb[c][:, n * 512 : (n + 1) * 512],
                        start=(c == 0),
                        stop=(c == 1),
                    )
                nc.vector.tensor_copy(QT_sb[m][:, n * 512 : (n + 1) * 512], ps)

            def kt_proj(m, q):
                # KT_sb[m][:, q*512:(q+1)*512]
                if KT_sb[m] is None:
                    KT_sb[m] = big.tile([P, S], f32r, name=f"KT{m}", tag=f"KT{m}")
                ps = psA.tile([P, 512], f32, name="ps_k", tag="psA")
                for c in range(2):
                    nc.tensor.matmul(
                        ps[:, :],
                        w_sb["wkt"][:, c, m * P : (m + 1) * P],
                        xT_sb[c][:, q * 512 : (q + 1) * 512],
                        start=(c == 0),
                        stop=(c == 1),
                    )
                nc.vector.tensor_copy(KT_sb[m][:, q * 512 : (q + 1) * 512], ps)

            def v_proj(mt):
                # V_aug in fp8, kt-PAIR layout for DoubleRow cd matmuls:
                # V8[j][:, i, h, 0:64] = V of key tile 2j+i, head h; col 64 is
                # a ones col (whose cd-matmul row is the softmax denominator).
                vt = big.tile([P, 4, 65], f32r, name=f"V{mt}", tag=f"V{mt}")
                ps = psA.tile([P, 512], f32, name="ps_v", tag="psA")
                for c in range(2):
                    nc.tensor.matmul(
                        ps[:, :D],
                        xT_sb[c][:, mt * P : (mt + 1) * P],
                        w_sb["wvt"][:, c, :],
                        start=(c == 0),
                        stop=(c == 1),
                    )
                nc.vector.tensor_copy(
                    vt[:, :, 0:64], ps[:, :D].rearrange("p (h e) -> p h e", h=4)
                )
                nc.vector.tensor_copy(vt[:, :, 64], ones4)
                V_sb[mt] = vt

            def kt_loop(p, f, inject=None):
                # rows 0-63: ctx.T for head 2p+h2; row 64: denominator.
                # One bank per head (h2 chooses the 512-col half).  The exp
                # tiles are written in fp8 kt-pair layout so each cd matmul
                # contracts TWO key tiles per pass (fp8 DoubleRow, 0.5
                # cycles/col): rhs free dims (2, 512) pick the kt-parity
                # halves, lhsT free dims (2, 65) the matching V slices.
                # Software-pipelined emission: the PE engine is in-order, so
                # cd(kt) is emitted only after scores(kt+2).  A cd matmul
                # blocked on its exp ACT (or, at kt=0, on the previous
                # section's cdsb eviction COPY) then always has two kt-steps
                # of independent scores work queued ahead of it, keeping the
                # PE busy (HAM clock gate stays released).
                ps_cd = psCD.tile([65, 1024], f32, name="ps_cd", tag="psCD")
                ets = {}

                def scores_act(kt):
                    ps_s = psA.tile([P, 1024], f32, name="ps_s", tag="psA")
                    for h2 in range(2):
                        nc.tensor.matmul(
                            ps_s[:, h2 * 512 : h2 * 512 + QH],
                            KT_sb[p][64 * h2 : 64 * h2 + 64, kt * P : (kt + 1) * P],
                            QT_sb[p][64 * h2 : 64 * h2 + 64, f * QH : (f + 1) * QH],
                            start=True,
                            stop=True,
                            tile_position=(64 * h2, 0),
                        )
                    et = etp.tile([P, 1024], f32r, name="et", tag="et")
                    nc.scalar.activation(
                        et, ps_s, Exp, bias=bias_sb[:, kt : kt + 1], scale=SCALE
                    )
                    ets[kt] = et

                scores_act(0)
                scores_act(1)
                for kt in range(NKT):
                    if kt + 2 < NKT:
                        scores_act(kt + 2)
                    et = ets.pop(kt)
                    for h2 in range(2):
                        h = 2 * p + h2
                        nc.tensor.matmul(
                            ps_cd[0:65, h2 * 512 : h2 * 512 + QH],
                            V_sb[kt][:, h, :],
                            et[:, h2 * 512 : h2 * 512 + QH],
                            start=(kt == 0),
                            stop=(kt == NKT - 1),
                        )
                    if inject and kt in inject:
                        inject[kt]()
                return ps_cd

            def finish_cd(p, f, ps_cd):
                # Evict ctx+den to SBUF right at section end, freeing the
                # PSUM slot for the next-but-one section.
                cdsb = big.tile([65, 1024], f32r, name=f"cd{p}{f}", tag=f"cd{p}{f}")
                nc.vector.tensor_copy(cdsb, ps_cd)
                cd_sb[(p, f)] = cdsb

            def den_recip(p, f):
                # Transpose the [1,1024] den row into partitions via 8
                # single-row matmuls, then one cheap [128,8] reciprocal.
                # col layout: h2*4 + q128 (q128 = 128-query block in half f).
                cdsb = cd_sb[(p, f)]
                ps_den = psA.tile([P, 16], f32, name="ps_den", tag="psA")
                for t in range(8):
                    h2, qq = t // 4, t % 4
                    # 2 duplicate output cols: fp32r ISA needs even free counts
                    nc.tensor.matmul(
                        ps_den[:, 2 * t : 2 * t + 2],
                        cdsb[64:65, h2 * 512 + qq * P : h2 * 512 + (qq + 1) * P],
                        one1[64:65, 0:2],
                        start=True,
                        stop=True,
                    )
                nc.vector.reciprocal(r_sb[(p, f)][:, 0:16], ps_den[:, 0:16])

            def r_ap(m, h):
                f, qq = m // 4, m % 4
                c = 2 * ((h % 2) * 4 + qq)
                return r_sb[(h // 2, f)][:, c : c + 1]

            def oproj_mm2(m, hpair):
                # per-head matmuls for heads (2*hpair, 2*hpair+1) of out tile m
                f, qq = m // 4, m % 4
                ps2 = psA.tile([P, 2, D], f32, name="ps2", tag="psA")
                for h2 in range(2):
                    h = 2 * hpair + h2
                    nc.tensor.matmul(
                        ps2[:, h2, :],
                        cd_sb[(hpair, f)][
                            0:64, h2 * 512 + qq * P : h2 * 512 + (qq + 1) * P
                        ],
                        wot_sb[:, h, :],
                        start=True,
                        stop=True,
                    )
                return ps2

            accA = {}

            def oproj_a(m):
                # tail tiles, heads 0-1: runs while the last section's cd
                # PSUM is still being evicted.  ScalarE (idle after the last
                # exp) does the 1/den scaling, GpSimd the add — DVE stays
                # free for the eviction COPY.
                ps2 = oproj_mm2(m, 0)
                t0 = small.tile([P, D], f32, name="tA", tag="tA")
                t1 = small.tile([P, D], f32, name="tA", tag="tA")
                nc.scalar.activation(t0, ps2[:, 0, :], Copy, scale=r_ap(m, 0))
                nc.scalar.activation(t1, ps2[:, 1, :], Copy, scale=r_ap(m, 1))
                acc = small.tile([P, D], f32, name=f"accA{m}", tag=f"accA{m}", bufs=1)
                nc.gpsimd.tensor_add(acc, t0, t1)
                accA[m] = acc

            def oproj_b(m):
                # tail tiles, heads 2-3 + final combine + output DMA
                ps2 = oproj_mm2(m, 1)
                t = small.tile([P, D], f32, name="acc", tag="acc")
                nc.vector.scalar_tensor_tensor(
                    t, ps2[:, 0, :], r_ap(m, 2), accA[m], Alu.mult, Alu.add
                )
                ot = small.tile([P, D], f32, name="ot", tag="ot", bufs=3)
                nc.vector.scalar_tensor_tensor(
                    ot, ps2[:, 1, :], r_ap(m, 3), t, Alu.mult, Alu.add
                )
                nc.sync.dma_start(out=out_d[m * P : (m + 1) * P, :], in_=ot)

            def oproj(m):
                # out tile m (queries m*128..): per-head matmul (no accum
                # across heads), then normalization folded into the combine:
                # out = sum_h ps4[:,h,:] * (1/den_h) with per-partition
                # scalars from r_sb.
                f, qq = m // 4, m % 4
                ps4 = psA.tile([P, 4, D], f32, name="ps4", tag="psA")
                for h in range(H):
                    p, h2 = h // 2, h % 2
                    nc.tensor.matmul(
                        ps4[:, h, :],
                        cd_sb[(p, f)][0:64, h2 * 512 + qq * P : h2 * 512 + (qq + 1) * P],
                        wot_sb[:, h, :],
                        start=True,
                        stop=True,
                    )

                acc = small.tile([P, D], f32, name="acc", tag="acc")
                nc.vector.tensor_scalar_mul(acc, ps4[:, 0, :], r_ap(m, 0))
                for h in range(1, H):
                    dst = (
                        small.tile([P, D], f32, name="acc", tag="acc")
                        if h < H - 1
                        else small.tile([P, D], f32, name="ot", tag="ot", bufs=3)
                    )
                    nc.vector.scalar_tensor_tensor(
                        dst, ps4[:, h, :], r_ap(m, h), acc, Alu.mult, Alu.add
                    )
                    acc = dst
                nc.sync.dma_start(out=out_d[m * P : (m + 1) * P, :], in_=acc)

            # ---- prologue: only what section (0,0) needs immediately ----
            qt_proj(0, 0)
            kt_proj(0, 0)
            for mt in range(3):
                v_proj(mt)

            # section (0,0): stream V tiles 2 steps ahead of their cd-use;
            # remaining K columns and pair-1 projections fill later steps.
            # NOTE: with the software-pipelined kt_loop, scores_act(j) is
            # EMITTED at loop iteration j-2; any injection producing KT
            # columns for score tile j must sit at kt <= j-3 (Tile deps
            # follow emission order).
            inj00 = {
                1: lambda: (v_proj(3), kt_proj(0, 1)),
                2: lambda: v_proj(4),
                3: lambda: v_proj(5),
                4: lambda: v_proj(6),
                5: lambda: (v_proj(7), kt_proj(0, 2)),
                6: lambda: v_proj(8),
                7: lambda: v_proj(9),
                8: lambda: (v_proj(10), kt_proj(0, 3)),
                9: lambda: v_proj(11),
                10: lambda: v_proj(12),
                11: lambda: (v_proj(13), qt_proj(1, 0)),
                12: lambda: v_proj(14),
                13: lambda: v_proj(15),
                14: lambda: kt_proj(1, 0),
            }
            cd00 = kt_loop(0, 0, inj00)
            finish_cd(0, 0, cd00)
            cd10 = kt_loop(
                1,
                0,
                {
                    1: lambda: kt_proj(1, 1),
                    3: lambda: qt_proj(0, 1),
                    5: lambda: kt_proj(1, 2),
                    7: lambda: den_recip(0, 0),
                    9: lambda: kt_proj(1, 3),
                },
            )
            finish_cd(1, 0, cd10)
            cd01 = kt_loop(
                0, 1, {3: lambda: qt_proj(1, 1), 6: lambda: den_recip(1, 0)}
            )
            finish_cd(0, 1, cd01)
            cd11 = kt_loop(
                1,
                1,
                {
                    3: lambda: oproj(0),
                    5: lambda: oproj(1),
                    7: lambda: den_recip(0, 1),
                    9: lambda: oproj(2),
                    11: lambda: oproj(3),
                },
            )
            finish_cd(1, 1, cd11)
            for m in range(4, 8):
                oproj_a(m)
            den_recip(1, 1)
            for m in range(4, 8):
                oproj_b(m)

    nc.compile()
    return nc


def _get_nc():
    if "nc" not in _cache:
        _cache["nc"] = _build_nc()
    return _cache["nc"]


def make_in_maps(x, W_Q, W_K, W_V, W_O, mask):
    wqt = np.ascontiguousarray(W_Q.T).astype(np.float32)
    wkt = np.ascontiguousarray(W_K.T).astype(np.float32)
    wvt = np.ascontiguousarray(W_V.T).astype(np.float32)
    wot = np.ascontiguousarray(W_O.T).astype(np.float32)
    in_maps = []
    for c in range(NCORES):
        b, qh = c // 2, c % 2
        xT_b = np.asarray(x[b]).T.astype(np.float32)
        xT_roll = np.ascontiguousarray(np.roll(xT_b, -qh * QS, axis=1))
        bias = np.where(np.asarray(mask[b]) == 0, -1e30, 0.0).astype(np.float32)
        bias = np.roll(bias, -qh * QS)
        bias = np.ascontiguousarray(bias.reshape(NKT, P).T)
        in_maps.append(
            {
                "xT": xT_roll,
                "wqt": wqt,
                "wkt": wkt,
                "wvt": wvt,
                "wot": wot,
                "bias": bias,
            }
        )
    return in_maps


def gather(results):
    out = np.empty((B, S, D), np.float32)
    for c in range(NCORES):
        b, qh = c // 2, c % 2
        out[b, qh * QS : (qh + 1) * QS, :] = results[c]["out"]
    return out


def kernel(x, W_Q, W_K, W_V, W_O, mask):
    from concourse.bass_utils import run_bass_kernel_spmd

    nc = _get_nc()
    in_maps = make_in_maps(x, W_Q, W_K, W_V, W_O, mask)
    res = run_bass_kernel_spmd(nc, in_maps, core_ids=list(range(NCORES)))
    return gather(res.results)
